# revision 1
# baseline (speedup 1.0000x reference)
"""DetectionLoss Bass/Tile kernel for TRN2 (one core = one image; SPMD x8).

Algorithm per core (image b):
  Phase A (j-loop over G=32 gts, split DVE/GPSIMD by j):
    per-anchor running max IoU (maxv) + 1-based argmax (bestp).
  Phase B (two column halves):
    pos = maxv >= 0.5; bestp_m = pos * bestp
    one-hot(bestp_m) -> PE transpose -> PE matmul vs block-diag gt table
      => gathered xg, yg, ln(wg), ln(hg), one-hot(label)*pos   (all pre-masked)
    smooth-L1 on encoded reg targets; focal via Exp/Ln pipeline.
  Output: [npos, sl1_sum, Nsum, corr] partial sums; host finishes.

Layout: anchor a <-> (partition p = a // COLS, col f = a % COLS).
Inputs (per core, planar, host-packed):
  anch [4, A] f32, clsp [8, A] f32, regp [4, A] f32, gtaux [1, 160] f32
  (gtaux = gx1[32] gy1[32] gx2[32] gy2[32] labelf[32])
Output: out [1, 4] f32.
"""
import dataclasses
import numpy as np

import concourse.bass as bass
import concourse.mybir as mybir
from concourse import tile

AL = mybir.AluOpType
AF = mybir.ActivationFunctionType
f32 = mybir.dt.float32

P = 128
G = 32
C = 8
EPS = 1e-7
BETA = 1.0 / 9.0
POS_IOU = 0.5


def patch_tile_drain(maxw: int = 1):
    """Split the TileContext exit drain's sem waits across NOPs (walrus
    setupSyncWait rejects >1 wait on a CTRL instruction in this build)."""
    import concourse.tile as tile_mod
    from concourse.vector_clock import ScopedClock

    def _drain_and_barrier(self, tick_clock, wait_clock):
        drain_inst = self.nc.sync.drain()
        wait_clock.add_sem_waits(
            drain_inst.ins, ScopedClock({None: tick_clock.global_clock})
        )
        si = drain_inst.ins.sync_info
        waits = list(si.on_wait)
        if len(waits) > maxw:
            si.on_wait = waits[:maxw]
            rest = waits[maxw:]
            for i in range(0, len(rest), maxw):
                nop = self.nc.sync.nop(nofuse=True, hint="drain_split")
                nop.ins.sync_info = mybir.SyncInfo(
                    on_wait=rest[i:i + maxw], on_update=[]
                )
        self.nc.all_engine_barrier()
        assert self.sems is not None
        popped = self.nc._tile_sem_poison_stack.pop()
        assert popped is self._sem_poison
        self.nc.clear_and_free_semaphores(list(self.sems.allocated().values()))
        self.nc.all_engine_barrier()

    tile_mod.TileContext._drain_and_barrier = _drain_and_barrier


def split_sync_waits(nc, maxw: int = 1):
    """Walrus rejects >2 sem waits on one instruction (and >1 on CTRL-type).
    Hoist excess waits onto same-engine NOPs inserted immediately before."""
    ctr = [0]

    def mknop(engine, waits):
        ctr[0] += 1
        nop = mybir.InstNoOp(name=f"I-wsplit-{ctr[0]}", ins=[], outs=[])
        nop.engine = engine
        nop.sync_info = mybir.SyncInfo(on_wait=waits, on_update=[])
        return nop

    for blk in nc.bb_map.values():
        bb = blk.bb
        il = bb.instructions
        i = 0
        while i < len(il):
            inst = il[i]
            si = inst.sync_info
            mw = 1 if isinstance(inst, mybir.InstTensorScalarPtr) else maxw
            if si is not None and len(si.on_wait) > mw:
                waits = list(si.on_wait)
                si.on_wait = waits[:mw]
                rest = waits[mw:]
                for k in range(0, len(rest), 1):
                    il.insert(i, mknop(inst.engine, rest[k:k + 1]))
                    i += 1
            i += 1


def _expand32(ap):
    """[P, n] AP -> [P, n, 32] with step-0 inner dim (broadcast)."""
    return dataclasses.replace(ap, ap=ap.ap + [[0, G]])


def build(A: int, cg: int = 640, logchain: bool = False, repeat: int = 1):
    """Emit the per-core program. A must be divisible by 128.
    cg: GPSIMD handles anchor-columns [0, cg); DVE handles [cg, COLS)."""
    assert A % P == 0
    COLS = A // P
    NSQ = (COLS + 15) // 16          # superquads (16 cols each)
    if NSQ % 2:
        NSQ += 1                      # want two equal halves
    W = NSQ * 16
    HS = NSQ // 2                     # superquads per half
    WH = W // 2                       # padded cols per half
    cg = min(cg, COLS)
    LN_THIRD = float(np.log(np.float32(1.0) / np.float32(3.0)))
    THIRD = float(np.float32(1.0) / np.float32(3.0))

    nc = bass.Bass()
    anch = nc.declare_dram_parameter("anch", [4, A], f32, isOutput=False)
    clsp = nc.declare_dram_parameter("clsp", [C, A], f32, isOutput=False)
    regp = nc.declare_dram_parameter("regp", [4, A], f32, isOutput=False)
    gtaux = nc.declare_dram_parameter("gtaux", [1, 5 * G], f32, isOutput=False)
    out = nc.declare_dram_parameter("out", [1, 4], f32, isOutput=True)

    def plane(t, c):
        # [n, A] dram plane c -> [P, COLS] AP
        return t[c].rearrange("(p w) -> p w", p=P)

    with tile.TileContext(nc) as tc:
        from contextlib import ExitStack
        for _rep in range(repeat):
          with ExitStack() as ctx:
              const = ctx.enter_context(tc.tile_pool(name="const", bufs=1))
              persist = ctx.enter_context(tc.tile_pool(name="persist", bufs=1))

              # ---------- constants ----------
              iotarep = const.tile([P, 512], f32, name="iotarep")
              nc.gpsimd.iota(iotarep[:], pattern=[[0, 16], [1, G]], base=1,
                             channel_multiplier=0,
                             allow_small_or_imprecise_dtypes=True)
              irow = const.tile([P, P], f32, name="irow")
              nc.gpsimd.iota(irow[:], pattern=[[1, P]], base=0,
                             channel_multiplier=0,
                             allow_small_or_imprecise_dtypes=True)
              icol = const.tile([P, 1], f32, name="icol")
              nc.gpsimd.iota(icol[:], pattern=[[0, 1]], base=0,
                             channel_multiplier=1,
                             allow_small_or_imprecise_dtypes=True)
              ident = const.tile([P, P], f32, name="ident")
              nc.vector.tensor_scalar(out=ident[:], in0=irow[:],
                                      scalar1=icol[:], scalar2=None,
                                      op0=AL.is_equal)
              ones = const.tile([P, 1], f32, name="ones")
              nc.gpsimd.memset(ones[:], 1.0)
              lnb = const.tile([P, 1], f32, name="lnb")
              nc.gpsimd.memset(lnb[:], 1e-30)

              # gt broadcast [P, 160]
              gtb = const.tile([P, 5 * G], f32, name="gtb")
              gsrc = gtaux[:]
              gsrc_b = dataclasses.replace(gsrc, ap=[[0, P]] + gsrc.ap[1:])
              nc.sync.dma_start(gtb[:], gsrc_b)
              # areaB [P, 32]
              wg_r = const.tile([P, G], f32, name="wg_r")
              nc.vector.tensor_tensor(out=wg_r[:], in0=gtb[:, 2 * G:3 * G],
                                      in1=gtb[:, 0:G], op=AL.subtract)
              hg_r = const.tile([P, G], f32, name="hg_r")
              nc.vector.tensor_tensor(out=hg_r[:], in0=gtb[:, 3 * G:4 * G],
                                      in1=gtb[:, G:2 * G], op=AL.subtract)
              areaB = const.tile([P, G], f32, name="areaB")
              nc.vector.tensor_tensor(out=areaB[:], in0=wg_r[:], in1=hg_r[:],
                                      op=AL.mult)

              # gather table, block-diagonal [P, 48]:
              # rows 32fs..32fs+32, cols 12fs..12fs+12 = [xg yg lwg lhg oh8]
              tt16 = const.tile([P, 48], f32, name="tt16")
              nc.gpsimd.memset(tt16[:], 0.0)
              traw = const.tile([G, 5], f32, name="traw")
              gsrc2 = dataclasses.replace(gsrc, ap=[[1, G], [G, 5]])
              nc.sync.dma_start(traw[:], gsrc2)
              tblk = const.tile([G, 12], f32, name="tblk")
              ttmp = const.tile([G, 1], f32, name="ttmp")
              # xg, yg
              nc.vector.tensor_tensor(out=ttmp[:], in0=traw[:, 0:1],
                                      in1=traw[:, 2:3], op=AL.add)
              nc.vector.tensor_scalar(out=tblk[:, 0:1], in0=ttmp[:],
                                      scalar1=0.5, scalar2=None, op0=AL.mult)
              nc.vector.tensor_tensor(out=ttmp[:], in0=traw[:, 1:2],
                                      in1=traw[:, 3:4], op=AL.add)
              nc.vector.tensor_scalar(out=tblk[:, 1:2], in0=ttmp[:],
                                      scalar1=0.5, scalar2=None, op0=AL.mult)
              # ln(wg), ln(hg)
              nc.vector.tensor_tensor(out=ttmp[:], in0=traw[:, 2:3],
                                      in1=traw[:, 0:1], op=AL.subtract)
              nc.scalar.activation(tblk[:, 2:3], ttmp[:], AF.Ln)
              nc.vector.tensor_tensor(out=ttmp[:], in0=traw[:, 3:4],
                                      in1=traw[:, 1:2], op=AL.subtract)
              nc.scalar.activation(tblk[:, 3:4], ttmp[:], AF.Ln)
              # one-hot(label)
              io8 = const.tile([G, C], f32, name="io8")
              nc.gpsimd.iota(io8[:], pattern=[[1, C]], base=0,
                             channel_multiplier=0,
                             allow_small_or_imprecise_dtypes=True)
              nc.vector.tensor_scalar(out=tblk[:, 4:12], in0=io8[:],
                                      scalar1=traw[:, 4:5], scalar2=None,
                                      op0=AL.is_equal)
              for fs in range(4):
                  nc.sync.dma_start(tt16[32 * fs:32 * fs + 32,
                                         12 * fs:12 * fs + 12], tblk[:])

              # ---------- anchors + per-anchor prep ----------
              pA_stack = ExitStack()
              pA = pA_stack.enter_context(tc.tile_pool(name="pA", bufs=1))
              ax1 = pA.tile([P, COLS], f32, name="ax1")
              ay1 = pA.tile([P, COLS], f32, name="ay1")
              ax2 = pA.tile([P, COLS], f32, name="ax2")
              ay2 = pA.tile([P, COLS], f32, name="ay2")
              for t, c in ((ax1, 0), (ay1, 1), (ax2, 2), (ay2, 3)):
                  nc.sync.dma_start(t[:], plane(anch, c))
              xa = persist.tile([P, COLS], f32, name="xa")
              ya = persist.tile([P, COLS], f32, name="ya")
              iwa = persist.tile([P, COLS], f32, name="iwa")
              iha = persist.tile([P, COLS], f32, name="iha")
              La = persist.tile([P, COLS], f32, name="La")
              Ha = persist.tile([P, COLS], f32, name="Ha")
              area_a = pA.tile([P, COLS], f32, name="area_a")
              prep_stack = ExitStack()
              prepp = prep_stack.enter_context(tc.tile_pool(name="prepp", bufs=1))
              wa_e = prepp.tile([P, COLS], f32, name="wa_e")
              ha_e = prepp.tile([P, COLS], f32, name="ha_e")
              nc.vector.scalar_tensor_tensor(out=wa_e[:], in0=ax2[:], scalar=EPS,
                                             in1=ax1[:], op0=AL.add,
                                             op1=AL.subtract)
              nc.vector.scalar_tensor_tensor(out=ha_e[:], in0=ay2[:], scalar=EPS,
                                             in1=ay1[:], op0=AL.add,
                                             op1=AL.subtract)
              nc.vector.scalar_tensor_tensor(out=area_a[:], in0=wa_e[:],
                                             scalar=-EPS, in1=ha_e[:],
                                             op0=AL.add, op1=AL.mult)
              nc.gpsimd.tensor_tensor(out=xa[:], in0=ax1[:], in1=ax2[:], op=AL.add)
              nc.gpsimd.tensor_scalar(out=xa[:], in0=xa[:], scalar1=0.5,
                                      scalar2=None, op0=AL.mult)
              nc.gpsimd.tensor_tensor(out=ya[:], in0=ay1[:], in1=ay2[:], op=AL.add)
              nc.gpsimd.tensor_scalar(out=ya[:], in0=ya[:], scalar1=0.5,
                                      scalar2=None, op0=AL.mult)
              nc.vector.reciprocal(iwa[:], wa_e[:])
              nc.vector.reciprocal(iha[:], ha_e[:])
              nc.scalar.activation(La[:], wa_e[:], AF.Ln)
              nc.scalar.activation(Ha[:], ha_e[:], AF.Ln)
              prep_stack.close()

              maxv = pA.tile([P, COLS], f32, name="maxv")
              bestp = pA.tile([P, COLS], f32, name="bestp")
              nc.vector.memset(maxv[:], -1e30)
              nc.vector.memset(bestp[:], 0.0)

              # ---------- phase A: j-loop, column-split ----------
              # GP cols [0, cg): GP computes inter/denom; ACT takes logs; DVE tail.
              # DVE cols [cg, COLS): full DVE chain with ACT relu.
              dw = COLS - cg
              with tc.tile_pool(name="jtmp", bufs=1) as jt:
                  if dw > 0:
                      d_ltx2 = [jt.tile([P, dw], f32, name=f"d_ltx{i}") for i in range(2)]
                      d_lty2 = [jt.tile([P, dw], f32, name=f"d_lty{i}") for i in range(2)]
                      d_wxr = [jt.tile([P, dw], f32, name=f"d_wxr{i}") for i in range(2)]
                      d_wxp = [jt.tile([P, dw], f32, name=f"d_wxp{i}") for i in range(2)]
                      d_wyr2 = [jt.tile([P, dw], f32, name=f"d_wyr{i}") for i in range(2)]
                      d_rd2 = (None if logchain else
                               [jt.tile([P, dw], f32, name=f"d_rd{i}") for i in range(2)])
                      d_upd2 = [jt.tile([P, dw], f32, name=f"d_upd{i}") for i in range(2)]
                      d_li = [jt.tile([P, dw], f32, name=f"d_li{i}") for i in range(2)]
                      d_ld = [jt.tile([P, dw], f32, name=f"d_ld{i}") for i in range(2)]
                      d_int2 = [jt.tile([P, dw], f32, name=f"d_int{i}") for i in range(3)]
                      d_den2 = [jt.tile([P, dw], f32, name=f"d_den{i}") for i in range(3)]
                  if cg > 0:
                      g_ltx = jt.tile([P, cg], f32, name="g_ltx")
                      g_lty = jt.tile([P, cg], f32, name="g_lty")
                      g_mnx = jt.tile([P, cg], f32, name="g_mnx")
                      g_wxr = [jt.tile([P, cg], f32, name=f"g_wxr{i}") for i in range(2)]
                      g_wxp = [jt.tile([P, cg], f32, name=f"g_wxp{i}") for i in range(2)]
                      g_wyr = [jt.tile([P, cg], f32, name=f"g_wyr{i}") for i in range(2)]
                      g_wyp = [jt.tile([P, cg], f32, name=f"g_wyp{i}") for i in range(2)]
                      g_ss = [jt.tile([P, cg], f32, name=f"g_ss{i}") for i in range(3)]
                      g_int = [jt.tile([P, cg], f32, name=f"g_int{i}") for i in range(3)]
                      g_li = [jt.tile([P, cg], f32, name=f"g_li{i}") for i in range(3)]
                      g_ld = [jt.tile([P, cg], f32, name=f"g_ld{i}") for i in range(3)]
                      g_lio = [jt.tile([P, cg], f32, name=f"g_lio{i}") for i in range(2)]
                      g_upd = jt.tile([P, cg], f32, name="g_upd")

                  def jstep_dve(j):
                      if dw == 0:
                          return
                      gx1 = gtb[:, j:j + 1]
                      gy1 = gtb[:, G + j:G + j + 1]
                      gx2 = gtb[:, 2 * G + j:2 * G + j + 1]
                      gy2 = gtb[:, 3 * G + j:3 * G + j + 1]
                      aB = areaB[:, j:j + 1]
                      s_ = slice(cg, COLS)
                      wxr = d_wxr[j % 2]
                      wxp = d_wxp[j % 2]
                      d_int = d_int2[j % 3]
                      d_den = d_den2[j % 3]
                      d_ltx = d_ltx2[j % 2]
                      d_lty = d_lty2[j % 2]
                      d_wyr = d_wyr2[j % 2]
                      d_rd = d_rd2[j % 2] if d_rd2 is not None else None
                      v = nc.vector
                      v.tensor_scalar(out=d_ltx[:], in0=ax1[:, s_], scalar1=gx1,
                                      scalar2=None, op0=AL.max)
                      v.scalar_tensor_tensor(out=wxr[:], in0=ax2[:, s_], scalar=gx2,
                                             in1=d_ltx[:], op0=AL.min,
                                             op1=AL.subtract)
                      nc.scalar.activation(wxp[:], wxr[:], AF.Relu)
                      v.tensor_scalar(out=d_lty[:], in0=ay1[:, s_], scalar1=gy1,
                                      scalar2=None, op0=AL.max)
                      v.scalar_tensor_tensor(out=d_wyr[:], in0=ay2[:, s_], scalar=gy2,
                                             in1=d_lty[:], op0=AL.min,
                                             op1=AL.subtract)
                      v.scalar_tensor_tensor(out=d_int[:], in0=d_wyr[:],
                                             scalar=0.0, in1=wxp[:],
                                             op0=AL.max, op1=AL.mult)
                      # Ssum = area_a + areaB_j (no inter dependency; on ACT)
                      nc.scalar.activation(d_den[:], area_a[:, s_], AF.Identity,
                                           bias=aB)
                      if logchain:
                          dli = d_li[j % 2]
                          dld = d_ld[j % 2]
                          nc.scalar.activation(dli[:], d_int[:], AF.Ln,
                                               bias=lnb[:])
                          nc.scalar.activation(dld[:], d_den[:], AF.Ln)
                          iou = d_lty  # log(t), t = inter/Ssum (monotone in iou)
                          v.scalar_tensor_tensor(out=iou[:], in0=dld[:],
                                                 scalar=-1.0, in1=dli[:],
                                                 op0=AL.mult, op1=AL.add)
                      else:
                          v.reciprocal(d_rd[:], d_den[:])
                          iou = d_lty  # t = inter/Ssum (monotone in iou)
                          v.tensor_tensor(out=iou[:], in0=d_int[:], in1=d_rd[:],
                                          op=AL.mult)
                      upd = d_upd2[j % 2]
                      v.tensor_tensor(out=upd[:], in0=iou[:], in1=maxv[:, s_],
                                      op=AL.is_gt)
                      v.tensor_tensor(out=maxv[:, s_], in0=maxv[:, s_], in1=iou[:],
                                      op=AL.max)
                      v.scalar_tensor_tensor(out=bestp[:, s_], in0=upd[:],
                                             scalar=float(j + 1), in1=bestp[:, s_],
                                             op0=AL.mult, op1=AL.max)

                  def jstep_gp(j):
                      if cg == 0:
                          return
                      gx1 = gtb[:, j:j + 1]
                      gy1 = gtb[:, G + j:G + j + 1]
                      gx2 = gtb[:, 2 * G + j:2 * G + j + 1]
                      gy2 = gtb[:, 3 * G + j:3 * G + j + 1]
                      aB = areaB[:, j:j + 1]
                      s_ = slice(0, cg)
                      gi_ = g_int[j % 3]
                      li = g_li[j % 3]
                      ld = g_ld[j % 3]
                      wxr = g_wxr[j % 2]; wxp = g_wxp[j % 2]
                      wyr = g_wyr[j % 2]; wyp = g_wyp[j % 2]
                      ss = g_ss[j % 3]
                      g = nc.gpsimd
                      g.tensor_scalar(out=g_ltx[:], in0=ax1[:, s_], scalar1=gx1,
                                      scalar2=None, op0=AL.max)
                      g.tensor_scalar(out=g_mnx[:], in0=ax2[:, s_], scalar1=gx2,
                                      scalar2=None, op0=AL.min)
                      g.tensor_tensor(out=wxr[:], in0=g_mnx[:], in1=g_ltx[:],
                                      op=AL.subtract)
                      nc.scalar.activation(wxp[:], wxr[:], AF.Relu)
                      g.tensor_scalar(out=g_lty[:], in0=ay1[:, s_], scalar1=gy1,
                                      scalar2=None, op0=AL.max)
                      g.tensor_scalar(out=g_mnx[:], in0=ay2[:, s_], scalar1=gy2,
                                      scalar2=None, op0=AL.min)
                      g.tensor_tensor(out=wyr[:], in0=g_mnx[:], in1=g_lty[:],
                                      op=AL.subtract)
                      nc.scalar.activation(wyp[:], wyr[:], AF.Relu)
                      g.tensor_tensor(out=gi_[:], in0=wxp[:], in1=wyp[:],
                                      op=AL.mult)
                      nc.scalar.activation(ss[:], area_a[:, s_], AF.Identity,
                                           bias=aB)
                      # logs on ACT; compare in log-t space (t = inter/Ssum)
                      nc.scalar.activation(li[:], gi_[:], AF.Ln, bias=lnb[:])
                      nc.scalar.activation(ld[:], ss[:], AF.Ln)

                  def jtail_gp(j):
                      if cg == 0:
                          return
                      s_ = slice(0, cg)
                      li = g_li[j % 3]
                      ld = g_ld[j % 3]
                      lio = g_lio[j % 2]
                      v = nc.vector
                      v.scalar_tensor_tensor(out=lio[:], in0=ld[:], scalar=-1.0,
                                             in1=li[:], op0=AL.mult, op1=AL.add)
                      v.tensor_tensor(out=g_upd[:], in0=lio[:], in1=maxv[:, s_],
                                      op=AL.is_gt)
                      v.tensor_tensor(out=maxv[:, s_], in0=maxv[:, s_],
                                      in1=lio[:], op=AL.max)
                      v.scalar_tensor_tensor(out=bestp[:, s_], in0=g_upd[:],
                                             scalar=float(j + 1), in1=bestp[:, s_],
                                             op0=AL.mult, op1=AL.max)

                  DELAY = 2
                  for j in range(G):
                      jstep_gp(j)
                      jstep_dve(j)
                      if j >= DELAY:
                          jtail_gp(j - DELAY)
                  for j in range(G - DELAY, G):
                      jtail_gp(j)

              # pos & masked bestp (padded to W); GP cols compare in log domain
              nposA = persist.tile([P, 1], f32, name="nposA")
              sl1A = persist.tile([P, 1], f32, name="sl1A")
              nsumA = persist.tile([P, 1], f32, name="nsumA")
              corrA = persist.tile([P, 1], f32, name="corrA")
              tacc = persist.tile([P, 1], f32, name="tacc")
              for t in (nposA, sl1A, nsumA, corrA):
                  nc.vector.memset(t[:], 0.0)
              pos = persist.tile([P, COLS], f32, name="pos")
              if cg > 0:
                  nc.vector.tensor_scalar(out=pos[:, 0:cg], in0=maxv[:, 0:cg],
                                          scalar1=LN_THIRD, scalar2=None,
                                          op0=AL.is_ge, op1=AL.add,
                                          accum_out=tacc[:])
                  nc.vector.tensor_tensor(out=nposA[:], in0=nposA[:],
                                          in1=tacc[:], op=AL.add)
              if COLS > cg:
                  thr = LN_THIRD if logchain else THIRD
                  nc.vector.tensor_scalar(out=pos[:, cg:COLS],
                                          in0=maxv[:, cg:COLS],
                                          scalar1=thr, scalar2=None,
                                          op0=AL.is_ge, op1=AL.add,
                                          accum_out=tacc[:])
                  nc.vector.tensor_tensor(out=nposA[:], in0=nposA[:],
                                          in1=tacc[:], op=AL.add)
              bpm = persist.tile([P, W], f32, name="bpm")
              nc.vector.memset(bpm[:], 0.0)
              nc.vector.tensor_tensor(out=bpm[:, 0:COLS], in0=pos[:],
                                      in1=bestp[:], op=AL.mult)
              pA_stack.close()

              # ---------- phase B ----------
              with ExitStack() as bctx:
                  ohp = bctx.enter_context(tc.tile_pool(name="ohp", bufs=2))
                  psum_t = bctx.enter_context(
                      tc.tile_pool(name="psum_t", bufs=2, space="PSUM"))
                  psum_g = bctx.enter_context(
                      tc.tile_pool(name="psum_g", bufs=2, space="PSUM"))
                  gath_p = bctx.enter_context(tc.tile_pool(name="gath", bufs=2))
                  scr = bctx.enter_context(tc.tile_pool(name="scr", bufs=1))
                  dmap = bctx.enter_context(tc.tile_pool(name="dmap", bufs=3))

                  sA1 = scr.tile([P, WH], f32, name="sA1")
                  sA2 = scr.tile([P, WH], f32, name="sA2")
                  sA3 = scr.tile([P, WH], f32, name="sA3")
                  sA4 = scr.tile([P, WH], f32, name="sA4")
                  sA5 = scr.tile([P, WH], f32, name="sA5")
                  sA6 = scr.tile([P, WH], f32, name="sA6")
                  sB1 = scr.tile([P, WH], f32, name="sB1")
                  sB2 = scr.tile([P, WH], f32, name="sB2")
                  sB3 = scr.tile([P, WH], f32, name="sB3")
                  sB4 = scr.tile([P, WH], f32, name="sB4")
                  sB5 = scr.tile([P, WH], f32, name="sB5")
                  sB6 = scr.tile([P, WH], f32, name="sB6")
                  s5 = sA5
                  fE = [scr.tile([P, WH], f32, name=f"fE{i}") for i in range(2)]
                  fU = [scr.tile([P, WH], f32, name=f"fU{i}") for i in range(2)]
                  fS = [scr.tile([P, WH], f32, name=f"fS{i}") for i in range(2)]
                  fG = [scr.tile([P, WH], f32, name=f"fG{i}") for i in range(2)]
                  fN = [scr.tile([P, WH], f32, name=f"fN{i}") for i in range(2)]
                  fP = [scr.tile([P, WH], f32, name=f"fP{i}") for i in range(2)]
                  Rp = [scr.tile([P, WH], f32, name=f"Rp{c}") for c in range(C)]

                  for half in range(2):
                      base = half * WH
                      rw = min(COLS - base, WH)   # real (unpadded) width
                      if rw <= 0:
                          break
                      gath = gath_p.tile([P, 12 * WH], f32, name="gath")

                      def gpl(m):
                          return gath[:, m * WH:m * WH + rw]

                      # gather: superquads
                      for s in range(HS):
                          sq = half * HS + s
                          oh = ohp.tile([P, 512], f32, name="oh")
                          src = _expand32(bpm[:, 16 * sq:16 * sq + 16])
                          nc.vector.tensor_tensor(
                              out=oh[:].rearrange("p (f j) -> p f j", j=G),
                              in0=src,
                              in1=iotarep[:].rearrange("p (f j) -> p f j", j=G),
                              op=AL.is_equal)
                          pt = psum_t.tile([P, 512], f32, name="pt")
                          for t4 in range(4):
                              nc.tensor.transpose(pt[:, 128 * t4:128 * t4 + 128],
                                                  oh[:, 128 * t4:128 * t4 + 128],
                                                  ident[:])
                          ohT = ohp.tile([P, 512], f32, name="ohT")
                          if s % 2 == 0:
                              nc.scalar.copy(ohT[:], pt[:])
                          else:
                              nc.vector.tensor_copy(ohT[:], pt[:])
                          gp = psum_g.tile([P, 192], f32, name="gp")
                          for t4 in range(4):
                              nc.tensor.matmul(out=gp[:, 48 * t4:48 * t4 + 48],
                                               lhsT=ohT[:, 128 * t4:128 * t4 + 128],
                                               rhs=tt16[:], start=True, stop=True)
                          # scatter copy psum -> planar gath slices
                          src_g = gp[:].rearrange("p (t f m) -> p t f m", t=4, f=4)
                          dst = gath[:]
                          dst_ap = dataclasses.replace(
                              dst, offset=dst.offset + 16 * s,
                              ap=[dst.ap[0], [4, 4], [1, 4], [WH, 12]])
                          nc.scalar.copy(dst_ap, src_g)

                      posh = pos[:, base:base + rw]
                      xah = xa[:, base:base + rw]
                      yah = ya[:, base:base + rw]
                      iwah = iwa[:, base:base + rw]
                      ihah = iha[:, base:base + rw]
                      Lah = La[:, base:base + rw]
                      Hah = Ha[:, base:base + rw]

                      # ---- reg: targets + smooth-L1 ----
                      for k, (gm, ctr, inv, lg) in enumerate(
                              ((0, xah, iwah, None), (1, yah, ihah, None),
                               (2, None, None, Lah), (3, None, None, Hah))):
                          s1, s2_, s3 = (sA1, sA2, sA3) if k % 2 == 0 else (sB1, sB2, sB3)
                          s4, s5, s6 = (sA4, sA5, sA6) if k % 2 == 0 else (sB4, sB5, sB6)
                          rt = s1
                          if lg is None:
                              nc.vector.tensor_tensor(out=s2_[:, :rw], in0=gpl(gm),
                                                      in1=ctr, op=AL.subtract)
                              nc.vector.tensor_tensor(out=rt[:, :rw], in0=s2_[:, :rw],
                                                      in1=inv, op=AL.mult)
                          else:
                              nc.vector.tensor_tensor(out=rt[:, :rw], in0=gpl(gm),
                                                      in1=lg, op=AL.subtract)
                          rp = dmap.tile([P, WH], f32, name="rp")
                          rsrc = plane(regp, k)
                          rsl = dataclasses.replace(
                              rsrc, offset=rsrc.offset + base,
                              ap=[rsrc.ap[0], [1, rw]])
                          nc.sync.dma_start(rp[:, :rw], rsl)
                          e = s2_
                          nc.vector.tensor_tensor(out=e[:, :rw], in0=rp[:, :rw],
                                                  in1=rt[:, :rw], op=AL.subtract)
                          q = s3
                          nc.scalar.activation(q[:, :rw], e[:, :rw], AF.Abs)
                          qm = s4
                          nc.gpsimd.tensor_tensor(out=qm[:, :rw], in0=q[:, :rw],
                                                  in1=posh, op=AL.mult)
                          cm = s5
                          nc.gpsimd.tensor_scalar(out=cm[:, :rw], in0=qm[:, :rw],
                                                  scalar1=BETA, scalar2=None,
                                                  op0=AL.min)
                          t2 = s6
                          nc.vector.scalar_tensor_tensor(out=t2[:, :rw],
                                                         in0=qm[:, :rw],
                                                         scalar=2.0,
                                                         in1=cm[:, :rw],
                                                         op0=AL.mult,
                                                         op1=AL.subtract)
                          nc.vector.scalar_tensor_tensor(out=e[:, :rw],
                                                         in0=cm[:, :rw],
                                                         scalar=0.0,
                                                         in1=t2[:, :rw],
                                                         op0=AL.add, op1=AL.mult,
                                                         accum_out=tacc[:])
                          nc.vector.tensor_tensor(out=sl1A[:], in0=sl1A[:],
                                                  in1=tacc[:], op=AL.add)

                      # ---- focal (gather-independent part) ----
                      for c in range(C):
                          xc = dmap.tile([P, WH], f32, name="xc")
                          csrc = plane(clsp, c)
                          csl = dataclasses.replace(
                              csrc, offset=csrc.offset + base,
                              ap=[csrc.ap[0], [1, rw]])
                          nc.sync.dma_start(xc[:, :rw], csl)
                          E = fE[c % 2]
                          nc.scalar.activation(E[:, :rw], xc[:, :rw], AF.Exp,
                                               scale=-1.0)
                          u = fU[c % 2]
                          nc.gpsimd.tensor_scalar(out=u[:, :rw], in0=E[:, :rw],
                                                  scalar1=1.0, scalar2=None,
                                                  op0=AL.add)
                          spn = fS[c % 2]
                          nc.scalar.activation(spn[:, :rw], u[:, :rw], AF.Ln)
                          sig = fG[c % 2]
                          nc.vector.reciprocal(sig[:, :rw], u[:, :rw])
                          sgn = fN[c % 2]
                          nc.gpsimd.tensor_tensor(out=sgn[:, :rw], in0=E[:, :rw],
                                                  in1=sig[:, :rw], op=AL.mult)
                          sp = fP[c % 2]
                          nc.gpsimd.tensor_tensor(out=sp[:, :rw], in0=xc[:, :rw],
                                                  in1=spn[:, :rw], op=AL.add)
                          s2t = E  # reuse: sig^2
                          nc.scalar.activation(s2t[:, :rw], sig[:, :rw], AF.Square)
                          Nt = sig  # N = sig^2 * sp  (overwrite sig)
                          nc.vector.scalar_tensor_tensor(out=Nt[:, :rw],
                                                         in0=s2t[:, :rw],
                                                         scalar=0.0,
                                                         in1=sp[:, :rw],
                                                         op0=AL.add, op1=AL.mult,
                                                         accum_out=tacc[:])
                          nc.vector.tensor_tensor(out=nsumA[:], in0=nsumA[:],
                                                  in1=tacc[:], op=AL.add)
                          s2n = sp  # reuse: sgn^2
                          nc.gpsimd.tensor_tensor(out=s2n[:, :rw], in0=sgn[:, :rw],
                                                  in1=sgn[:, :rw], op=AL.mult)
                          Pt = fU[c % 2]  # P = sgn^2 * spn
                          nc.gpsimd.tensor_tensor(out=Pt[:, :rw], in0=s2n[:, :rw],
                                                  in1=spn[:, :rw], op=AL.mult)
                          nc.vector.scalar_tensor_tensor(out=Rp[c][:, :rw],
                                                         in0=Pt[:, :rw],
                                                         scalar=1.0 / 3.0,
                                                         in1=Nt[:, :rw],
                                                         op0=AL.mult,
                                                         op1=AL.subtract)
                      # ---- corr dots (need gather) ----
                      for c in range(C):
                          s5c = sA5 if c % 2 == 0 else sB5
                          nc.gpsimd.tensor_tensor(out=s5c[:, :rw],
                                                  in0=gpl(4 + c),
                                                  in1=Rp[c][:, :rw], op=AL.mult)
                          nc.scalar.activation(s5c[:, :rw], s5c[:, :rw],
                                               AF.Identity, accum_out=tacc[:])
                          nc.vector.tensor_tensor(out=corrA[:], in0=corrA[:],
                                                  in1=tacc[:], op=AL.add)

              # ---------- final cross-partition reduce ----------
              acc4 = persist.tile([P, 4], f32, name="acc4")
              nc.scalar.copy(acc4[:, 0:1], nposA[:])
              nc.scalar.copy(acc4[:, 1:2], sl1A[:])
              nc.scalar.copy(acc4[:, 2:3], nsumA[:])
              nc.scalar.copy(acc4[:, 3:4], corrA[:])
              with tc.tile_pool(name="psum_f", bufs=1, space="PSUM") as pf:
                  fps = pf.tile([1, 4], f32, name="fps")
                  nc.tensor.matmul(out=fps[:], lhsT=ones[:], rhs=acc4[:],
                                   start=True, stop=True)
                  osb = persist.tile([1, 4], f32, name="osb")
                  nc.scalar.copy(osb[:], fps[:])
                  nc.sync.dma_start(out[:], osb[:])

    return nc


# ---------------- host side ----------------

def pack_inputs(cls_preds, reg_preds, anchors, gt_boxes, gt_labels):
    """Full inputs -> list of 8 per-core input maps (planar layouts)."""
    B, A, _ = cls_preds.shape
    anch = np.ascontiguousarray(anchors.astype(np.float32).T)         # [4, A]
    maps = []
    for b in range(B):
        clsp = np.ascontiguousarray(cls_preds[b].astype(np.float32).T)  # [8, A]
        regp = np.ascontiguousarray(reg_preds[b].astype(np.float32).T)  # [4, A]
        gb = gt_boxes[b].astype(np.float32)
        lab = gt_labels[b].astype(np.float32)
        gtaux = np.concatenate([gb[:, 0], gb[:, 1], gb[:, 2], gb[:, 3],
                                lab]).astype(np.float32)[None, :]       # [1,160]
        maps.append({"anch": anch, "clsp": clsp, "regp": regp,
                     "gtaux": gtaux})
    return maps


def finish(partials):
    """partials: list of [1,4] arrays per core -> (cls_loss, reg_loss)."""
    f = np.float32
    npos = f(0); sl1 = f(0); nsum = f(0); corr = f(0)
    for p in partials:
        p = p.reshape(4)
        npos += f(p[0]); sl1 += f(p[1]); nsum += f(p[2]); corr += f(p[3])
    denom = max(float(npos), 1.0)
    if npos > 0:
        cls_loss = f(0.75) * (nsum + corr) / f(denom)
        reg_loss = sl1 / f(2 * BETA) / f(denom)
    else:
        cls_loss = f(0.0); reg_loss = f(0.0)
    return np.float32(cls_loss), np.float32(reg_loss)


# ---------------- self-contained kernel entry ----------------

_CACHE = {}

def _get_fn(n_cores=8):
    """Build + jit the 8-core SPMD executable once."""
    if "fn" in _CACHE:
        return _CACHE["fn"]
    import jax
    from jax.sharding import Mesh, PartitionSpec, NamedSharding
    from jax.experimental.shard_map import shard_map
    from concourse.bass2jax import (_bass_exec_p, install_neuronx_cc_hook,
                                    partition_id_tensor)
    patch_tile_drain(1)
    nc = build(160000, cg=512, logchain=True)
    split_sync_waits(nc)
    install_neuronx_cc_hook()
    in_names, out_names, out_avals, zero_shapes = [], [], [], []
    partition_name = (nc.partition_id_tensor.name
                      if nc.partition_id_tensor else None)
    for alloc in nc.m.functions[0].allocations:
        if not isinstance(alloc, mybir.MemoryLocationSet):
            continue
        name = alloc.memorylocations[0].name
        if alloc.kind == "ExternalInput":
            if name != partition_name:
                in_names.append(name)
        elif alloc.kind == "ExternalOutput":
            out_names.append(name)
            shape = tuple(alloc.tensor_shape)
            dtype = mybir.dt.np(alloc.dtype)
            out_avals.append(jax.core.ShapedArray(shape, dtype))
            zero_shapes.append((shape, dtype))
    n_params = len(in_names)
    n_outs = len(out_avals)
    all_in_names = in_names + out_names + ([partition_name]
                                           if partition_name else [])
    donate = tuple(range(n_params, n_params + n_outs))

    def _body(*args):
        operands = list(args)
        if partition_name is not None:
            operands.append(partition_id_tensor())
        outs = _bass_exec_p.bind(
            *operands, out_avals=tuple(out_avals),
            in_names=tuple(all_in_names), out_names=tuple(out_names),
            lowering_input_output_aliases=(),
            sim_require_finite=True, sim_require_nnan=True, nc=nc)
        return tuple(outs)

    devices = jax.devices()[:n_cores]
    mesh = Mesh(np.asarray(devices), ("core",))
    in_specs = (PartitionSpec("core"),) * (n_params + n_outs)
    out_specs = (PartitionSpec("core"),) * len(out_names)
    fn = jax.jit(shard_map(_body, mesh=mesh, in_specs=in_specs,
                           out_specs=out_specs, check_rep=False),
                 donate_argnums=donate, keep_unused=True)
    sh = NamedSharding(mesh, PartitionSpec("core"))
    _CACHE["fn"] = (fn, in_names, out_names, out_avals, zero_shapes, sh,
                    n_cores)
    return _CACHE["fn"]


def kernel(cls_preds, reg_preds, anchors, gt_boxes, gt_labels):
    """Full-input DetectionLoss on 8 NeuronCores (data-parallel over batch).

    Returns (cls_loss, reg_loss) as float32 scalars, matching reference()."""
    import jax
    cls_preds = np.asarray(cls_preds)
    reg_preds = np.asarray(reg_preds)
    anchors = np.asarray(anchors)
    gt_boxes = np.asarray(gt_boxes)
    gt_labels = np.asarray(gt_labels)
    B, A, _ = cls_preds.shape
    assert (B, A) == (8, 160000), (B, A)
    maps = pack_inputs(cls_preds, reg_preds, anchors, gt_boxes, gt_labels)
    fn, in_names, out_names, out_avals, zero_shapes, sh, n_cores = _get_fn()
    concat_in = [jax.device_put(
        np.concatenate([np.asarray(maps[c][nm]) for c in range(n_cores)],
                       axis=0), sh) for nm in in_names]
    zeros = [jax.device_put(
        np.zeros((n_cores * s[0], *s[1:]), d), sh) for s, d in zero_shapes]
    out_arrs = fn(*concat_in, *zeros)
    res = np.asarray(out_arrs[out_names.index("out")]).reshape(n_cores, 1, 4)
    partials = [res[c] for c in range(n_cores)]
    cls_loss, reg_loss = finish(partials)
    return cls_loss, reg_loss



# revision 38
# speedup vs baseline: 1.5740x; 1.5740x over previous
"""DetectionLoss Bass/Tile kernel for TRN2 (one core = one image; SPMD x8).

v2 design (bit-packed argmax, f16 clamp geometry):

Phase A per gt j over [P, COLS] (half-scaled f16 coords):
  ltx = clamp(ax1, [gx1_j, gx2_j])     ts
  ux  = clamp(ax2, [gx1_j, gx2_j])     ts      wxp = ux - ltx  (exact relu'd overlap)
  lty, vy                              ts x2   wyn = lty - vy  (= -wyp)
  interneg = wxp * wyn                 tt      (= -inter, <= 0; exact w/ containment)
  n4n = interneg + aB3C_j + areaA3q    stt     (positive; min over j <=> best score)
  Q = (bits(n4n) & ~255) | (lab_j*32+j)  ts int32
  M = min(M, Q)                        tt int32 (parity-split chains)
Decode: pos = M <= bits(CBIG); j* = M & 31; label = (M >> 5) & 7.
Gather: one-hot(j*) per j via ts is_equal (bf16), PE transpose, block-diag
  bf16 hi/lo gt-table matmul -> xg/yg/lwg/lhg planes; smooth-L1 vs reg_preds.
Focal: per class exp/ln chain, nsum via ttr accumulate; x[a,label] selected
  via copy_predicated; corr = sum pos * R(x_lab).
Output [1,4]: [npos, sl1_sum(*2beta), nsum, corr]; host finishes.
"""
import dataclasses
import numpy as np

import concourse.bass as bass
import concourse.mybir as mybir
from concourse import tile

AL = mybir.AluOpType
AF = mybir.ActivationFunctionType
f32 = mybir.dt.float32
f16 = mybir.dt.float16
bf16 = mybir.dt.bfloat16
i32 = mybir.dt.int32
i16 = mybir.dt.int16

P = 128
G = 32
C = 8
EPS = 1e-7
BETA = 1.0 / 9.0
CBIG = float(2.0 ** 17)
CBIG_BITS = int(np.float32(CBIG).view(np.int32))

# gtf rows (f32): half-scaled gt coords + quarter-scaled area/3 + CBIG,
# then bf16 hi/lo payload splits (full-scale values).
RF_GX1, RF_GY1, RF_GX2, RF_GY2, RF_AB3C = 0, 1, 2, 3, 4
RF_XGH, RF_XGL, RF_YGH, RF_YGL = 5, 6, 7, 8
RF_LWH, RF_LWL, RF_LHH, RF_LHL = 9, 10, 11, 12
NF = 13
# gti rows (int32): lj = lab*32 + j; jiota = j
RI_LJ, RI_J = 0, 1
NI = 2


def patch_tile_drain(maxw: int = 1):
    import concourse.tile as tile_mod
    from concourse.vector_clock import ScopedClock

    def _drain_and_barrier(self, tick_clock, wait_clock):
        drain_inst = self.nc.sync.drain()
        wait_clock.add_sem_waits(
            drain_inst.ins, ScopedClock({None: tick_clock.global_clock})
        )
        si = drain_inst.ins.sync_info
        waits = list(si.on_wait)
        if len(waits) > maxw:
            si.on_wait = waits[:maxw]
            rest = waits[maxw:]
            for i in range(0, len(rest), maxw):
                nop = self.nc.sync.nop(nofuse=True, hint="drain_split")
                nop.ins.sync_info = mybir.SyncInfo(
                    on_wait=rest[i:i + maxw], on_update=[]
                )
        self.nc.all_engine_barrier()
        assert self.sems is not None
        popped = self.nc._tile_sem_poison_stack.pop()
        assert popped is self._sem_poison
        self.nc.clear_and_free_semaphores(list(self.sems.allocated().values()))
        self.nc.all_engine_barrier()

    tile_mod.TileContext._drain_and_barrier = _drain_and_barrier


def split_sync_waits(nc, maxw: int = 1):
    ctr = [0]

    def mknop(engine, waits):
        ctr[0] += 1
        nop = mybir.InstNoOp(name=f"I-wsplit-{ctr[0]}", ins=[], outs=[])
        nop.engine = engine
        nop.sync_info = mybir.SyncInfo(on_wait=waits, on_update=[])
        return nop

    for blk in nc.bb_map.values():
        bb = blk.bb
        il = bb.instructions
        i = 0
        while i < len(il):
            inst = il[i]
            si = inst.sync_info
            mw = 1 if isinstance(inst, mybir.InstTensorScalarPtr) else maxw
            if si is not None and len(si.on_wait) > mw:
                waits = list(si.on_wait)
                si.on_wait = waits[:mw]
                rest = waits[mw:]
                for k in range(0, len(rest), 1):
                    il.insert(i, mknop(inst.engine, rest[k:k + 1]))
                    i += 1
            i += 1


def build(A: int, pool_n4=True, pool_min_par=0, debug=False, GCH=16):
    """Emit the per-core program. A must be divisible by 128.
    pool_n4: put n4n stt on Pool engine.
    pool_min_par: of 4 runmin parity chains, how many go to Pool."""
    assert A % P == 0
    COLS = A // P
    NSQ = (COLS + 15) // 16
    WG = NSQ * 16                  # gather width (padded to superquads)
    nc = bass.Bass()
    anch = nc.declare_dram_parameter("anch", [4, A], f32, isOutput=False)
    clsp = nc.declare_dram_parameter("clsp", [C, A], f16, isOutput=False)
    regp = nc.declare_dram_parameter("regp", [4, A], f16, isOutput=False)
    gtf = nc.declare_dram_parameter("gtf", [1, NF * G], f32, isOutput=False)
    gti = nc.declare_dram_parameter("gti", [1, NI * G], i32, isOutput=False)
    out = nc.declare_dram_parameter("out", [1, 4], f32, isOutput=True)
    if debug:
        dbg = nc.declare_dram_parameter("dbg", [P, 4 * COLS], f32, isOutput=True)

    def plane(t, c):
        return t[c].rearrange("(p w) -> p w", p=P)

    with tile.TileContext(nc) as tc:
        from contextlib import ExitStack
        with ExitStack() as ctx:
            const = ctx.enter_context(tc.tile_pool(name="const", bufs=1))
            persist = ctx.enter_context(tc.tile_pool(name="persist", bufs=1))

            # ---------- constants ----------
            irow = const.tile([P, P], f32, name="irow")
            nc.gpsimd.iota(irow[:], pattern=[[1, P]], base=0,
                           channel_multiplier=0,
                           allow_small_or_imprecise_dtypes=True)
            icol = const.tile([P, 1], f32, name="icol")
            nc.gpsimd.iota(icol[:], pattern=[[0, 1]], base=0,
                           channel_multiplier=1,
                           allow_small_or_imprecise_dtypes=True)
            identB = const.tile([P, P], bf16, name="identB")
            nc.vector.tensor_scalar(out=identB[:], in0=irow[:],
                                    scalar1=icol[:], scalar2=None,
                                    op0=AL.is_equal)
            ones = const.tile([P, 1], f32, name="ones")
            nc.gpsimd.memset(ones[:], 1.0)

            # gt broadcasts
            gtfb = const.tile([P, NF * G], f32, name="gtfb")
            gsrc = gtf[:]
            nc.sync.dma_start(
                gtfb[:], dataclasses.replace(gsrc, ap=[[0, P]] + gsrc.ap[1:]))
            gtib = const.tile([P, NI * G], i32, name="gtib")
            gisrc = gti[:]
            nc.sync.dma_start(
                gtib[:], dataclasses.replace(gisrc, ap=[[0, P]] + gisrc.ap[1:]))

            def gf(r, j):
                return gtfb[:, r * G + j:r * G + j + 1]

            def gint(r, j):
                return gtib[:, r * G + j:r * G + j + 1]

            # payload table (bf16): tblk [G, 8] = [xgh xgl ygh ygl lwh lwl lhh lhl]
            # rows RF_XGH..RF_LHL are contiguous -> one strided DMA
            tblk32 = const.tile([G, 8], f32, name="tblk32")
            src = dataclasses.replace(gsrc, offset=gsrc.offset + RF_XGH * G,
                                      ap=[[1, G], [G, 8]])
            nc.sync.dma_start(tblk32[:], src)
            tblk = const.tile([G, 8], bf16, name="tblk")
            nc.vector.tensor_copy(tblk[:], tblk32[:])
            ttb = const.tile([P, 32], bf16, name="ttb")
            nc.gpsimd.memset(ttb[:], 0.0)
            for cg in range(4):
                nc.sync.dma_start(ttb[32 * cg:32 * cg + 32,
                                      8 * cg:8 * cg + 8], tblk[:])

            # xc pool opened early: must be below axp on the pool stack
            fnsum_stack = ExitStack()
            xc_pool = fnsum_stack.enter_context(
                tc.tile_pool(name="xcp", bufs=1))

            # ---------- anchors + prep ----------
            axh_stack = ExitStack()
            axp = axh_stack.enter_context(tc.tile_pool(name="axp", bufs=1))
            pA_stack = ExitStack()
            pA = pA_stack.enter_context(tc.tile_pool(name="pA", bufs=1))
            ax = [pA.tile([P, COLS], f32, name=f"ax{i}") for i in range(4)]
            for i in range(4):
                nc.sync.dma_start(ax[i][:], plane(anch, i))
            axh = [axp.tile([P, COLS], f16, name=f"axh{i}") for i in range(4)]
            for i in range(4):
                nc.vector.tensor_scalar(out=axh[i][:], in0=ax[i][:],
                                        scalar1=0.5, scalar2=None, op0=AL.mult)
            wa = pA.tile([P, COLS], f32, name="wa")
            ha = pA.tile([P, COLS], f32, name="ha")
            nc.vector.tensor_tensor(out=wa[:], in0=ax[2][:], in1=ax[0][:],
                                    op=AL.subtract)
            nc.vector.tensor_tensor(out=ha[:], in0=ax[3][:], in1=ax[1][:],
                                    op=AL.subtract)
            negareaA3 = persist.tile([P, COLS], f32, name="negareaA3")
            nc.vector.scalar_tensor_tensor(out=negareaA3[:], in0=wa[:],
                                           scalar=-1.0 / 12.0, in1=ha[:],
                                           op0=AL.mult, op1=AL.mult)
            ca = persist.tile([P, COLS], f16, name="ca")
            ya = persist.tile([P, COLS], f16, name="ya")
            t0 = pA.tile([P, COLS], f32, name="t0")
            nc.vector.tensor_tensor(out=t0[:], in0=ax[0][:], in1=ax[2][:],
                                    op=AL.add)
            nc.vector.tensor_scalar(out=ca[:], in0=t0[:], scalar1=0.5,
                                    scalar2=None, op0=AL.mult)
            nc.vector.tensor_tensor(out=t0[:], in0=ax[1][:], in1=ax[3][:],
                                    op=AL.add)
            nc.vector.tensor_scalar(out=ya[:], in0=t0[:], scalar1=0.5,
                                    scalar2=None, op0=AL.mult)
            wae = pA.tile([P, COLS], f32, name="wae")
            hae = pA.tile([P, COLS], f32, name="hae")
            nc.vector.tensor_scalar(out=wae[:], in0=wa[:], scalar1=EPS,
                                    scalar2=None, op0=AL.add)
            nc.vector.tensor_scalar(out=hae[:], in0=ha[:], scalar1=EPS,
                                    scalar2=None, op0=AL.add)
            iwa32 = pA.tile([P, COLS], f32, name="iwa32")
            iha32 = pA.tile([P, COLS], f32, name="iha32")
            nc.vector.reciprocal(iwa32[:], wae[:])
            nc.vector.reciprocal(iha32[:], hae[:])
            iwa = persist.tile([P, COLS], f16, name="iwa")
            iha = persist.tile([P, COLS], f16, name="iha")
            nc.vector.tensor_copy(iwa[:], iwa32[:])
            nc.vector.tensor_copy(iha[:], iha32[:])
            La = persist.tile([P, COLS], f16, name="La")
            Ha = persist.tile([P, COLS], f16, name="Ha")
            nc.scalar.activation(La[:], wae[:], AF.Ln)
            nc.scalar.activation(Ha[:], hae[:], AF.Ln)
            pA_stack.close()

            nposA = persist.tile([P, 1], f32, name="nposA")
            sl1A = persist.tile([P, 1], f32, name="sl1A")
            nsumA = persist.tile([P, 1], f32, name="nsumA")
            corrA = persist.tile([P, 1], f32, name="corrA")
            tacc = persist.tile([P, 1], f32, name="tacc")
            tacc2 = persist.tile([P, 1], f32, name="tacc2")
            for t in (nposA, sl1A, nsumA, corrA):
                nc.vector.memset(t[:], 0.0)

            # ---------- focal nsum (pos-independent; interleaved w/ phase A)
            xcs = [xc_pool.tile([P, COLS], f16, name=f"xc{c}")
                   for c in range(C)]
            fns_stack = ExitStack()
            fs1 = fns_stack.enter_context(tc.tile_pool(name="fns", bufs=1))
            fEe = [fs1.tile([P, COLS], f16, name=f"Ee{i}") for i in range(2)]
            fuu = [fs1.tile([P, COLS], f16, name=f"uu{i}") for i in range(2)]
            fsn = [fs1.tile([P, COLS], f16, name=f"sn{i}") for i in range(2)]
            fsg = [fs1.tile([P, COLS], f16, name=f"sg{i}") for i in range(2)]
            fs2 = [fs1.tile([P, COLS], f16, name=f"s2{i}") for i in range(2)]
            fsp = [fs1.tile([P, COLS], f16, name=f"sp{i}") for i in range(2)]
            fNo = [fs1.tile([P, COLS], f16, name=f"No{i}") for i in range(2)]

            def emit_focal_class(c):
                xc = xcs[c]
                nc.sync.dma_start(xc[:], plane(clsp, c))
                i2 = c % 2
                nc.scalar.activation(fEe[i2][:], xc[:], AF.Exp, scale=-1.0)
                nc.scalar.activation(fuu[i2][:], fEe[i2][:], AF.Identity,
                                     bias=1.0)
                nc.scalar.activation(fsn[i2][:], fuu[i2][:], AF.Ln)
                nc.scalar.activation(fsg[i2][:], fsn[i2][:], AF.Exp,
                                     scale=-1.0)
                nc.scalar.activation(fs2[i2][:], fsg[i2][:], AF.Square)
                nc.vector.tensor_tensor(out=fsp[i2][:], in0=xc[:],
                                        in1=fsn[i2][:], op=AL.add)
                nc.vector.scalar_tensor_tensor(
                    out=fNo[i2][:], in0=fs2[i2][:], scalar=0.0,
                    in1=fsp[i2][:], op0=AL.add, op1=AL.mult,
                    accum_out=tacc[:])
                nc.vector.tensor_tensor(out=nsumA[:], in0=nsumA[:],
                                        in1=tacc[:], op=AL.add)

            # ---------- phase A ----------
            NCH = 4                       # runmin parity chains
            mch_stack = ExitStack()
            mchp = mch_stack.enter_context(tc.tile_pool(name="mch", bufs=1))
            # M chains kept as f32: packed ints are positive, so float min
            # == int min, and Pool lacks int32 min.
            M = [persist.tile([P, COLS], f32, name="M0")] + [
                mchp.tile([P, COLS], f32, name=f"M{k}") for k in range(1, NCH)]
            for k in range(NCH):
                nc.vector.memset(M[k][:], 1e30)
            with tc.tile_pool(name="jtmp", bufs=1) as jt:
                ltx2 = [jt.tile([P, COLS], f16, name=f"ltx{i}") for i in range(2)]
                ux2 = [jt.tile([P, COLS], f16, name=f"ux{i}") for i in range(2)]
                lty2 = [jt.tile([P, COLS], f16, name=f"lty{i}") for i in range(2)]
                vy2 = [jt.tile([P, COLS], f16, name=f"vy{i}") for i in range(2)]
                wxp2 = [jt.tile([P, COLS], f16, name=f"wxp{i}") for i in range(2)]
                wyn2 = [jt.tile([P, COLS], f16, name=f"wyn{i}") for i in range(2)]
                int2 = [jt.tile([P, COLS], f16, name=f"int{i}") for i in range(3)]
                n4f = [jt.tile([P, COLS], f32, name=f"n4f{i}") for i in range(3)]
                qt = [jt.tile([P, COLS], f32, name=f"qt{i}") for i in range(3)]
                for j in range(G):
                    ltx = ltx2[j % 2]; ux = ux2[j % 2]
                    lty = lty2[j % 2]; vy = vy2[j % 2]
                    wxp = wxp2[j % 2]; wyn = wyn2[j % 2]
                    itg = int2[j % 3]; n4 = n4f[j % 3]; q = qt[j % 3]
                    v = nc.vector
                    v.tensor_scalar(out=ltx[:], in0=axh[0][:],
                                    scalar1=gf(RF_GX1, j), scalar2=gf(RF_GX2, j),
                                    op0=AL.max, op1=AL.min)
                    v.tensor_scalar(out=ux[:], in0=axh[2][:],
                                    scalar1=gf(RF_GX2, j), scalar2=gf(RF_GX1, j),
                                    op0=AL.min, op1=AL.max)
                    v.tensor_scalar(out=lty[:], in0=axh[1][:],
                                    scalar1=gf(RF_GY1, j), scalar2=gf(RF_GY2, j),
                                    op0=AL.max, op1=AL.min)
                    v.tensor_scalar(out=vy[:], in0=axh[3][:],
                                    scalar1=gf(RF_GY2, j), scalar2=gf(RF_GY1, j),
                                    op0=AL.min, op1=AL.max)
                    v.tensor_tensor(out=wxp[:], in0=ux[:], in1=ltx[:],
                                    op=AL.subtract)
                    v.tensor_tensor(out=wyn[:], in0=lty[:], in1=vy[:],
                                    op=AL.subtract)
                    v.tensor_tensor(out=itg[:], in0=wxp[:], in1=wyn[:],
                                    op=AL.mult)
                    # areaA3 is j-independent: applied at decode instead.
                    nc.scalar.activation(n4[:], itg[:], AF.Identity,
                                         bias=gf(RF_AB3C, j))
                    v.tensor_scalar(out=q[:].bitcast(i32),
                                    in0=n4[:].bitcast(i32),
                                    scalar1=-256, scalar2=gint(RI_LJ, j),
                                    op0=AL.bitwise_and, op1=AL.bitwise_or)
                    k = j % NCH
                    eng_min = nc.gpsimd if (k < pool_min_par) else nc.vector
                    eng_min.tensor_tensor(out=M[k][:], in0=M[k][:],
                                          in1=q[:], op=AL.min)
                    if j % 4 == 3:
                        emit_focal_class(j // 4)
            # merge parity chains
            nc.vector.tensor_tensor(out=M[0][:], in0=M[0][:], in1=M[1][:],
                                    op=AL.min)
            nc.vector.tensor_tensor(out=M[2][:], in0=M[2][:], in1=M[3][:],
                                    op=AL.min)
            nc.vector.tensor_tensor(out=M[0][:], in0=M[0][:], in1=M[2][:],
                                    op=AL.min)
            Qmin = M[0]
            mch_stack.close()
            fns_stack.close()
            axh_stack.close()

            # ---------- decode ----------
            pos = persist.tile([P, COLS], f16, name="pos")
            nc.vector.scalar_tensor_tensor(out=pos[:], in0=Qmin[:],
                                           scalar=-CBIG, in1=negareaA3[:],
                                           op0=AL.add, op1=AL.is_le)
            jbf = persist.tile([P, COLS], bf16, name="jbf")
            labh = persist.tile([P, COLS], f16, name="labh")
            with tc.tile_pool(name="dec", bufs=1) as decp:
                jdi = decp.tile([P, COLS], i32, name="jdi")
                nc.vector.tensor_scalar(out=jdi[:], in0=Qmin[:].bitcast(i32),
                                        scalar1=31,
                                        scalar2=None, op0=AL.bitwise_and)
                nc.vector.tensor_copy(jbf[:], jdi[:])
                lbi = decp.tile([P, COLS], i32, name="lbi")
                nc.vector.tensor_scalar(out=lbi[:], in0=Qmin[:].bitcast(i32),
                                        scalar1=5,
                                        scalar2=7, op0=AL.arith_shift_right,
                                        op1=AL.bitwise_and)
                nc.vector.tensor_copy(labh[:], lbi[:])

            pscr = persist.tile([P, COLS], f16, name="pscr")
            nc.scalar.activation(pscr[:], pos[:], AF.Identity,
                                 accum_out=nposA[:])

            if debug:
                dview = dbg[:].rearrange("p (n w) -> p n w", n=4)
                nc.scalar.copy(dview[:, 0, :], pos[:])
                nc.vector.tensor_copy(dview[:, 1, :], jbf[:])
                nc.scalar.copy(dview[:, 2, :], labh[:])
                nc.vector.tensor_copy(dview[:, 3, :], Qmin[:])

            # ---------- gather ----------
            gath = persist.tile([P, 8, WG], bf16, name="gath")
            with ExitStack() as gctx:
                ohp = gctx.enter_context(tc.tile_pool(name="ohp", bufs=2))
                psum_t = gctx.enter_context(
                    tc.tile_pool(name="psum_t", bufs=2, space="PSUM"))
                psum_g = gctx.enter_context(
                    tc.tile_pool(name="psum_g", bufs=2, space="PSUM"))
                for ch0 in range(0, NSQ, GCH):
                    nsq = min(GCH, NSQ - ch0)
                    wch = nsq * 16
                    c0 = ch0 * 16
                    rw = min(COLS - c0, wch)      # real cols in chunk
                    oh = ohp.tile([P, wch, G], bf16, name="oh")
                    if rw < wch:
                        pad = oh[:, rw:wch, :].rearrange("p a b -> p (a b)")
                        nc.vector.memset(pad, 0.0)
                    for j in range(G):
                        nc.vector.tensor_scalar(
                            out=oh[:, 0:rw, j],
                            in0=jbf[:, c0:c0 + rw],
                            scalar1=float(j), scalar2=None, op0=AL.is_equal)
                    ohf = oh[:].rearrange("p a b -> p (a b)")
                    for s in range(nsq):
                        pt = psum_t.tile([P, 512], bf16, name="pt")
                        for t4 in range(4):
                            nc.tensor.transpose(
                                pt[:, 128 * t4:128 * t4 + 128],
                                ohf[:, 512 * s + 128 * t4:512 * s + 128 * t4 + 128],
                                identB[:])
                        ohT = ohp.tile([P, 512], bf16, name="ohT")
                        nc.vector.tensor_copy(ohT[:], pt[:])
                        gp = psum_g.tile([P, 128], f32, name="gp")
                        for t4 in range(4):
                            nc.tensor.matmul(
                                out=gp[:, 32 * t4:32 * t4 + 32],
                                lhsT=ohT[:, 128 * t4:128 * t4 + 128],
                                rhs=ttb[:], start=True, stop=True)
                        # gp [P, (t4, cg4, pl8)] -> gath [P, pl, col=c0+16s+4t4+cg]
                        src = gp[:].rearrange("p (t c m) -> p t c m", t=4, c=4)
                        dst = gath[:]
                        dst_ap = dataclasses.replace(
                            dst, offset=dst.offset + (c0 + 16 * s),
                            ap=[dst.ap[0], [4, 4], [1, 4], [WG, 8]])
                        nc.scalar.copy(dst_ap, src)

            # ---------- reg smooth-L1 ----------
            with ExitStack() as rctx:
                rp_p = rctx.enter_context(tc.tile_pool(name="rp", bufs=2))
                rs = rctx.enter_context(tc.tile_pool(name="rs", bufs=1))
                gv = [rs.tile([P, COLS], f16, name=f"gv{i}") for i in range(2)]
                rt = [rs.tile([P, COLS], f16, name=f"rt{i}") for i in range(2)]
                ef = [rs.tile([P, COLS], f16, name=f"ef{i}") for i in range(2)]
                qf = [rs.tile([P, COLS], f16, name=f"qf{i}") for i in range(2)]
                qm = [rs.tile([P, COLS], f16, name=f"qm{i}") for i in range(2)]
                cm = [rs.tile([P, COLS], f16, name=f"cm{i}") for i in range(2)]
                q2 = [rs.tile([P, COLS], f16, name=f"q2{i}") for i in range(2)]
                t2 = [rs.tile([P, COLS], f16, name=f"t2{i}") for i in range(2)]
                so = [rs.tile([P, COLS], f16, name=f"so{i}") for i in range(2)]
                for k, (hi, lo, ctr, inv, lg) in enumerate((
                        (0, 1, ca, iwa, None), (2, 3, ya, iha, None),
                        (4, 5, None, None, La), (6, 7, None, None, Ha))):
                    i2 = k % 2
                    nc.vector.tensor_tensor(out=gv[i2][:], in0=gath[:, hi, 0:COLS],
                                            in1=gath[:, lo, 0:COLS], op=AL.add)
                    if lg is None:
                        nc.vector.tensor_tensor(out=gv[i2][:], in0=gv[i2][:],
                                                in1=ctr[:], op=AL.subtract)
                        nc.vector.tensor_tensor(out=rt[i2][:], in0=gv[i2][:],
                                                in1=inv[:], op=AL.mult)
                    else:
                        nc.vector.tensor_tensor(out=rt[i2][:], in0=gv[i2][:],
                                                in1=lg[:], op=AL.subtract)
                    rp = rp_p.tile([P, COLS], f16, name="rp")
                    nc.sync.dma_start(rp[:], plane(regp, k))
                    e = ef[i2]
                    nc.vector.tensor_tensor(out=e[:], in0=rp[:], in1=rt[i2][:],
                                            op=AL.subtract)
                    nc.scalar.activation(qf[i2][:], e[:], AF.Abs)
                    nc.vector.tensor_tensor(out=qm[i2][:], in0=qf[i2][:],
                                            in1=pos[:], op=AL.mult)
                    nc.vector.tensor_scalar(out=cm[i2][:], in0=qm[i2][:],
                                            scalar1=BETA, scalar2=None,
                                            op0=AL.min)
                    nc.vector.tensor_scalar(out=q2[i2][:], in0=qm[i2][:],
                                            scalar1=2.0, scalar2=None,
                                            op0=AL.mult)
                    nc.vector.tensor_tensor(out=t2[i2][:], in0=q2[i2][:],
                                            in1=cm[i2][:], op=AL.subtract)
                    nc.vector.scalar_tensor_tensor(
                        out=so[i2][:], in0=cm[i2][:], scalar=0.0,
                        in1=t2[i2][:], op0=AL.add, op1=AL.mult,
                        accum_out=tacc2[:])
                    nc.vector.tensor_tensor(out=sl1A[:], in0=sl1A[:],
                                            in1=tacc2[:], op=AL.add)

            # ---------- focal ----------
            with ExitStack() as fctx:
                fs = fctx.enter_context(tc.tile_pool(name="fs", bufs=1))
                x_lab = fs.tile([P, COLS], f16, name="x_lab")
                nc.vector.memset(x_lab[:], 0.0)
                mk = [fs.tile([P, COLS], i16, name=f"mk{i}") for i in range(2)]
                for c in range(C):
                    i2 = c % 2
                    nc.vector.tensor_scalar(out=mk[i2][:], in0=labh[:],
                                            scalar1=float(c), scalar2=None,
                                            op0=AL.is_equal)
                    nc.vector.copy_predicated(out=x_lab[:], mask=mk[i2][:],
                                              data=xcs[c][:])

                # R(x_lab) chain
                El = fs.tile([P, COLS], f16, name="El")
                ul = fs.tile([P, COLS], f16, name="ul")
                snl = fs.tile([P, COLS], f16, name="snl")
                sgl = fs.tile([P, COLS], f16, name="sgl")
                s2l = fs.tile([P, COLS], f16, name="s2l")
                spl = fs.tile([P, COLS], f16, name="spl")
                Nl = fs.tile([P, COLS], f16, name="Nl")
                sganl = fs.tile([P, COLS], f16, name="sganl")
                s2nl = fs.tile([P, COLS], f16, name="s2nl")
                Pl = fs.tile([P, COLS], f16, name="Pl")
                Rl = fs.tile([P, COLS], f16, name="Rl")
                Ro = fs.tile([P, COLS], f16, name="Ro")
                nc.scalar.activation(El[:], x_lab[:], AF.Exp, scale=-1.0)
                nc.scalar.activation(ul[:], El[:], AF.Identity, bias=1.0)
                nc.scalar.activation(snl[:], ul[:], AF.Ln)
                nc.scalar.activation(sgl[:], snl[:], AF.Exp, scale=-1.0)
                nc.scalar.activation(s2l[:], sgl[:], AF.Square)
                nc.vector.tensor_tensor(out=spl[:], in0=x_lab[:], in1=snl[:],
                                        op=AL.add)
                nc.vector.tensor_tensor(out=Nl[:], in0=s2l[:], in1=spl[:],
                                        op=AL.mult)
                nc.vector.tensor_tensor(out=sganl[:], in0=El[:], in1=sgl[:],
                                        op=AL.mult)
                nc.scalar.activation(s2nl[:], sganl[:], AF.Square)
                nc.vector.tensor_tensor(out=Pl[:], in0=s2nl[:], in1=snl[:],
                                        op=AL.mult)
                nc.vector.scalar_tensor_tensor(out=Rl[:], in0=Pl[:],
                                               scalar=1.0 / 3.0, in1=Nl[:],
                                               op0=AL.mult, op1=AL.subtract)
                nc.vector.scalar_tensor_tensor(
                    out=Ro[:], in0=Rl[:], scalar=0.0, in1=pos[:],
                    op0=AL.add, op1=AL.mult, accum_out=tacc[:])
                nc.vector.tensor_tensor(out=corrA[:], in0=corrA[:],
                                        in1=tacc[:], op=AL.add)
            fnsum_stack.close()

            # ---------- final reduce ----------
            acc4 = persist.tile([P, 4], f32, name="acc4")
            nc.scalar.copy(acc4[:, 0:1], nposA[:])
            nc.scalar.copy(acc4[:, 1:2], sl1A[:])
            nc.scalar.copy(acc4[:, 2:3], nsumA[:])
            nc.scalar.copy(acc4[:, 3:4], corrA[:])
            with tc.tile_pool(name="psum_f", bufs=1, space="PSUM") as pf:
                fps = pf.tile([1, 4], f32, name="fps")
                nc.tensor.matmul(out=fps[:], lhsT=ones[:], rhs=acc4[:],
                                 start=True, stop=True)
                osb = persist.tile([1, 4], f32, name="osb")
                nc.scalar.copy(osb[:], fps[:])
                nc.sync.dma_start(out[:], osb[:])

    return nc


# ---------------- host side ----------------

def pack_inputs(cls_preds, reg_preds, anchors, gt_boxes, gt_labels):
    B, A, _ = cls_preds.shape
    anch = np.ascontiguousarray(anchors.astype(np.float32).T)
    f = np.float32
    maps = []
    import ml_dtypes
    bfc = lambda v: f(f(v).astype(ml_dtypes.bfloat16))
    for b in range(B):
        clsp = np.ascontiguousarray(cls_preds[b].astype(np.float16).T)
        regp = np.ascontiguousarray(reg_preds[b].astype(np.float16).T)
        gb = gt_boxes[b].astype(f)
        lab = gt_labels[b].astype(np.int64)
        gx1, gy1, gx2, gy2 = gb[:, 0], gb[:, 1], gb[:, 2], gb[:, 3]
        aB3C = (gx2 - gx1) * (gy2 - gy1) * f(1.0 / 12.0) + f(CBIG)
        xg = (gx1 + gx2) * f(0.5)
        yg = (gy1 + gy2) * f(0.5)
        lwg = np.log(gx2 - gx1, dtype=f)
        lhg = np.log(gy2 - gy1, dtype=f)
        rows = np.zeros((NF, G), f)
        rows[RF_GX1] = gx1 * f(0.5)
        rows[RF_GY1] = gy1 * f(0.5)
        rows[RF_GX2] = gx2 * f(0.5)
        rows[RF_GY2] = gy2 * f(0.5)
        rows[RF_AB3C] = aB3C
        for rh, rl, v in ((RF_XGH, RF_XGL, xg), (RF_YGH, RF_YGL, yg),
                          (RF_LWH, RF_LWL, lwg), (RF_LHH, RF_LHL, lhg)):
            h = bfc(v)
            rows[rh] = h
            rows[rl] = bfc(v - h)
        irows = np.zeros((NI, G), np.int32)
        irows[RI_LJ] = (lab.astype(np.int32) * 32
                        + np.arange(G, dtype=np.int32))
        irows[RI_J] = np.arange(G, dtype=np.int32)
        maps.append({"anch": anch, "clsp": clsp, "regp": regp,
                     "gtf": rows.reshape(1, -1),
                     "gti": irows.reshape(1, -1)})
    return maps


def finish(partials):
    f = np.float32
    npos = f(0); sl1 = f(0); nsum = f(0); corr = f(0)
    for p in partials:
        p = p.reshape(4)
        npos += f(p[0]); sl1 += f(p[1]); nsum += f(p[2]); corr += f(p[3])
    denom = max(float(npos), 1.0)
    if npos > 0:
        cls_loss = f(0.75) * (nsum + corr) / f(denom)
        reg_loss = sl1 / f(2 * BETA) / f(denom)
    else:
        cls_loss = f(0.0); reg_loss = f(0.0)
    return np.float32(cls_loss), np.float32(reg_loss)


# ---------------- self-contained kernel entry ----------------

_CACHE = {}


def _get_fn(n_cores=8):
    if "fn" in _CACHE:
        return _CACHE["fn"]
    import jax
    from jax.sharding import Mesh, PartitionSpec, NamedSharding
    from jax.experimental.shard_map import shard_map
    from concourse.bass2jax import (_bass_exec_p, install_neuronx_cc_hook,
                                    partition_id_tensor)
    patch_tile_drain(1)
    nc = build(160000)
    split_sync_waits(nc)
    install_neuronx_cc_hook()
    in_names, out_names, out_avals, zero_shapes = [], [], [], []
    partition_name = (nc.partition_id_tensor.name
                      if nc.partition_id_tensor else None)
    for alloc in nc.m.functions[0].allocations:
        if not isinstance(alloc, mybir.MemoryLocationSet):
            continue
        name = alloc.memorylocations[0].name
        if alloc.kind == "ExternalInput":
            if name != partition_name:
                in_names.append(name)
        elif alloc.kind == "ExternalOutput":
            out_names.append(name)
            shape = tuple(alloc.tensor_shape)
            dtype = mybir.dt.np(alloc.dtype)
            out_avals.append(jax.core.ShapedArray(shape, dtype))
            zero_shapes.append((shape, dtype))
    n_params = len(in_names)
    n_outs = len(out_avals)
    all_in_names = in_names + out_names + ([partition_name]
                                           if partition_name else [])
    donate = tuple(range(n_params, n_params + n_outs))

    def _body(*args):
        operands = list(args)
        if partition_name is not None:
            operands.append(partition_id_tensor())
        outs = _bass_exec_p.bind(
            *operands, out_avals=tuple(out_avals),
            in_names=tuple(all_in_names), out_names=tuple(out_names),
            lowering_input_output_aliases=(),
            sim_require_finite=True, sim_require_nnan=True, nc=nc)
        return tuple(outs)

    devices = jax.devices()[:n_cores]
    mesh = Mesh(np.asarray(devices), ("core",))
    in_specs = (PartitionSpec("core"),) * (n_params + n_outs)
    out_specs = (PartitionSpec("core"),) * len(out_names)
    fn = jax.jit(shard_map(_body, mesh=mesh, in_specs=in_specs,
                           out_specs=out_specs, check_rep=False),
                 donate_argnums=donate, keep_unused=True)
    sh = NamedSharding(mesh, PartitionSpec("core"))
    _CACHE["fn"] = (fn, in_names, out_names, out_avals, zero_shapes, sh,
                    n_cores)
    return _CACHE["fn"]


def kernel(cls_preds, reg_preds, anchors, gt_boxes, gt_labels):
    import jax
    cls_preds = np.asarray(cls_preds)
    reg_preds = np.asarray(reg_preds)
    anchors = np.asarray(anchors)
    gt_boxes = np.asarray(gt_boxes)
    gt_labels = np.asarray(gt_labels)
    B, A, _ = cls_preds.shape
    assert (B, A) == (8, 160000), (B, A)
    maps = pack_inputs(cls_preds, reg_preds, anchors, gt_boxes, gt_labels)
    fn, in_names, out_names, out_avals, zero_shapes, sh, n_cores = _get_fn()
    concat_in = [jax.device_put(
        np.concatenate([np.asarray(maps[c][nm]) for c in range(n_cores)],
                       axis=0), sh) for nm in in_names]
    zeros = [jax.device_put(
        np.zeros((n_cores * s[0], *s[1:]), d), sh) for s, d in zero_shapes]
    out_arrs = fn(*concat_in, *zeros)
    res = np.asarray(out_arrs[out_names.index("out")]).reshape(n_cores, 1, 4)
    partials = [res[c] for c in range(n_cores)]
    cls_loss, reg_loss = finish(partials)
    return cls_loss, reg_loss
